# revision 70
# baseline (speedup 1.0000x reference)
"""Distributed causal attention (RoPE, QKV/out projections) on 8 TRN2 NeuronCores.

Sharding: tensor-parallel over heads. H=16 heads -> 2 heads per core.
Each core:
  - reads the full (transposed, bf16) activations xT [D, B*S]
  - computes qT/kT for its 2 heads (column-sharded wq/wk), applies RoPE
  - computes v in natural [s, hd] orientation (for the PV matmul lhsT),
    augmented with a ones-column so PV also produces the softmax denominator
  - flash-style causal attention with scores kept transposed [sk, sq] so
    softmax reduction runs on the TensorEngine via the ones-column trick
  - row-sharded output projection -> partial output [B, D, S]
Host sums the 8 partials and transposes back to [B, S, D].

Schedule: all QKV-projection and output-projection work is emitted as
"filler" pieces interleaved into the attention tile loops, so the PE has
dense work while ACT streams the softmax exps.  Chunk order places the
large (high-j) attention chunks mid-kernel where filler is plentiful and
small 4-tile chunks at the start/end.
"""

import numpy as np
import ml_dtypes

import concourse.mybir as mybir
from concourse import bacc
import concourse.tile as tile
from concourse.bass import ts, ds

B, S, D, H, HD = 2, 2048, 1024, 16, 64
NCORES = 8
HL = H // NCORES            # heads per core = 2
EL = HL * HD                # local e-dims per core = 128
BS = B * S                  # 4096
DCH = D // 128              # 8 contraction chunks
NCHUNK = BS // 512          # 8 projection chunks (both batches)
SQJ = S // 512              # 4 q-chunks per batch
NKT = S // 128              # 16 k-tiles per batch
THETA = 10000.0
BF = mybir.dt.bfloat16
F32 = mybir.dt.float32
FP8 = mybir.dt.float8e4
DR = mybir.MatmulPerfMode.DoubleRow
EXPFN = mybir.ActivationFunctionType.Exp
CPYFN = mybir.ActivationFunctionType.Copy
WSCALE = 16.0               # fp8 weights are stored x16; undone in rope/v-copy

_nc_cache = {}


def build_nc(debug=False):
    key = bool(debug)
    if key in _nc_cache:
        return _nc_cache[key]
    nc = bacc.Bacc("TRN2", target_bir_lowering=False, debug=debug, num_devices=NCORES)

    xT_d = nc.dram_tensor("xT", [D, BS], BF, kind="ExternalInput")
    cos_d = nc.dram_tensor("cosT", [128, S], BF, kind="ExternalInput")
    sin_d = nc.dram_tensor("sinT", [128, S], BF, kind="ExternalInput")
    wq_d = nc.dram_tensor("wqT", [D, EL], BF, kind="ExternalInput")
    wk_d = nc.dram_tensor("wkT", [D, EL], BF, kind="ExternalInput")
    perm_d = nc.dram_tensor("permT", [128, 128], BF, kind="ExternalInput")
    wv_d = nc.dram_tensor("wvT", [D, EL], BF, kind="ExternalInput")
    wo_d = nc.dram_tensor("woT", [EL, D], BF, kind="ExternalInput")
    mi_d = nc.dram_tensor("maskI", [128, 64], BF, kind="ExternalInput")
    su_d = nc.dram_tensor("stepU", [128, 2, 128], BF, kind="ExternalInput")
    out_d = nc.dram_tensor("out", [B, D, S], BF, kind="ExternalOutput")

    with tile.TileContext(nc) as tc:
        with (
            tc.tile_pool(name="sb", bufs=1) as sb,
            tc.tile_pool(name="work", bufs=2) as work,
            tc.tile_pool(name="ps", bufs=1, space="PSUM") as ps,
        ):
            # ---- persistent SBUF tensors ----
            xts = sb.tile([128, DCH, BS], BF)      # x transposed, d on partitions
            wqs = sb.tile([128, DCH, EL], BF)
            wks = sb.tile([128, DCH, EL], BF)
            perms = sb.tile([128, 128], BF)
            wvs = sb.tile([128, DCH, EL], BF)
            wos = sb.tile([128, D], BF)
            coss = sb.tile([128, S], BF)
            sins = sb.tile([128, S], BF)
            qt = sb.tile([128, BS], BF)
            kt = sb.tile([128, BS], BF)
            vaug = sb.tile([128, B * NKT, 130], BF)  # per k-tile: [v_h0|1|v_h1|1]
            attnT = sb.tile([128, BS], BF)           # normalized attn out, heads stacked

            # ---- input DMAs: weights + first x column groups spread across
            # queues so chunk 0's projection can start almost immediately ----
            mi64s = sb.tile([128, 64], BF)       # -240 * [p%64 == m]
            steps = sb.tile([128, 2, 128], BF)   # [(p%64) + 64s > q]

            for k in range(DCH):
                nc.sync.dma_start(out=wqs[:, k, :], in_=wq_d[ts(k, 128), :])
            nc.gpsimd.dma_start(out=wks[:], in_=wk_d[:, :].rearrange("(k p) e -> p k e", p=128))

            # x loads in three granularities: chunk-0 columns first (unblocks
            # the first projection fast), then the rest of batch 0, then
            # batch 1 — contiguous per-partition rows keep descriptors large.
            def xcols(k, lo, n, eng):
                eng.dma_start(
                    out=xts[:, k, ds(lo, n)], in_=xT_d[ts(k, 128), ds(lo, n)]
                )

            for k in range(DCH):
                xcols(k, 0, 512, (nc.sync, nc.gpsimd, nc.scalar)[k % 3])
            nc.sync.dma_start(out=perms[:], in_=perm_d[:, :])
            nc.sync.dma_start(out=mi64s[:], in_=mi_d[:, :])
            nc.sync.dma_start(out=steps[:], in_=su_d[:, :, :])
            for k in range(DCH):
                xcols(k, 512, 512, (nc.gpsimd, nc.sync, nc.scalar)[k % 3])
            nc.gpsimd.dma_start(out=coss[:], in_=cos_d[:, :])
            nc.gpsimd.dma_start(out=sins[:], in_=sin_d[:, :])
            for k in range(DCH):
                xcols(k, 1024, 1024, (nc.sync, nc.gpsimd)[k % 2])
            nc.gpsimd.dma_start(out=wvs[:], in_=wv_d[:, :].rearrange("(k p) e -> p k e", p=128))
            for k in range(DCH):
                xcols(k, S, S, (nc.sync, nc.gpsimd)[k % 2])
            nc.gpsimd.dma_start(out=wos[:], in_=wo_d[:, :])

            # ones columns for the PV denominator rows (full memset also marks
            # the tensor initialized for the simulator's strided-AP reads)
            nc.gpsimd.memset(vaug[:], 1.0)
            ones64 = sb.tile([1, 64], BF)
            nc.vector.memset(ones64[:], 1.0)
            scratch1 = sb.tile([128, 1], F32)
            nc.vector.memset(scratch1[:], 0.0)
            nc.scalar.activation(scratch1[:], scratch1[:], EXPFN)  # preload exp table

            # ---- QKV projection pieces for one 512-col chunk of B*S.
            # Returns a list of closures, each a contiguous PE work unit that
            # can be dropped into an attention tile loop as filler. ----
            def proj_pieces(c):
                cs = ds(c * 512, 512)
                scol = ds((c % SQJ) * 512, 512)     # position columns within batch
                pieces = []

                def qk(wtile, rot, nm):
                    st = {}

                    def mms():
                        pp = ps.tile([128, 512], F32, tag="work", bufs=2, name=f"pp{nm}{c}")
                        for k in range(DCH):
                            nc.tensor.matmul(
                                pp[:], wtile[:, k, :], xts[:, k, cs],
                                start=(k == 0), stop=(k == DCH - 1),
                            )
                        raw = work.tile([128, 512], BF, tag="rawt", bufs=2, name="raw")
                        nc.vector.tensor_copy(raw[:], pp[:])
                        st["raw"] = raw

                    def ropep():
                        raw = st["raw"]
                        sp2 = ps.tile([128, 512], F32, tag="work", bufs=2, name="sp2")
                        nc.tensor.matmul(sp2[:], perms[:], raw[:], start=True, stop=True)
                        rtmp = work.tile([128, 512], BF, tag="ropetmp", bufs=2, name="rtmp")
                        nc.vector.tensor_mul(rot[:, cs], raw[:], coss[:, scol])
                        nc.vector.tensor_mul(rtmp[:], sp2[:], sins[:, scol])
                        nc.vector.tensor_add(rot[:, cs], rot[:, cs], rtmp[:])

                    return [mms, ropep]

                pieces += qk(wqs, qt, "q")
                pieces += qk(wks, kt, "k")

                def vtile(st4):
                    def go():
                        t128 = c * 4 + st4
                        vp = ps.tile([128, 512], F32, tag="work", bufs=2, name=f"vp{t128}")
                        for k in range(DCH):
                            nc.tensor.matmul(
                                vp[:, 0:128], xts[:, k, ds(t128 * 128, 128)], wvs[:, k, :],
                                start=(k == 0), stop=(k == DCH - 1),
                            )
                        dst = vaug[:, t128, :].rearrange("p (g y) -> p g y", g=2)[:, :, 0:64]
                        src = vp[:, 0:128].rearrange("p (g y) -> p g y", g=2)
                        if st4 % 2 == 0:
                            nc.scalar.copy(dst, src)
                        else:
                            nc.vector.tensor_copy(dst, src)
                    return go

                pieces += [vtile(st4) for st4 in range(4)]
                return pieces

            # ---- output projection pieces for one (batch, q-chunk) ----
            def oproj_pieces(b, j, tail=False):
                oc = ds(b * S + j * 512, 512)
                ost = work.tile([128, 8, 512], BF, tag="ostage", bufs=2, name="ost")

                def piece(e):
                    def go():
                        op = ps.tile([128, 512], F32, tag="work", bufs=2, name="op")
                        nc.tensor.matmul(
                            op[:], wos[:, ts(e, 128)], attnT[:, oc],
                            start=True, stop=True,
                        )
                        # tail chunks alternate copy engines so the 2-bank op
                        # ring drains at two-copies-in-flight rate; elsewhere
                        # keep ACT exp-only
                        act = (e % 2 == 1) if tail else False
                        if act:
                            nc.scalar.copy(ost[:, e, :], op[:])
                        else:
                            nc.vector.tensor_copy(ost[:, e, :], op[:])
                    return go

                def store():
                    nc.sync.dma_start(
                        out=out_d[b].rearrange("(ec p) s -> p ec s", p=128)[:, :, ts(j, 512)],
                        in_=ost[:],
                    )
                return [piece(e) for e in range(8)] + [store]

            # ---- causal attention for one (batch, 512-wide q-chunk), with
            # filler pieces spread across the tile loop ----
            def attn_chunk(b, j, fillers=(), last=False):
                fillers = list(fillers)
                ntk = 4 * (j + 1)
                if last:
                    # allocate the PV accumulator from the sc ring: avoids the
                    # cross-chunk stall on the single-buffered pv ring (the sc
                    # serialization this causes is cheap in a 4-tile chunk)
                    pvt = ps.tile([128, 2, 512], F32, tag="sc", bufs=2, name="pvlast")
                    pv = pvt[:].rearrange("p a b -> p (a b)")
                else:
                    pv = ps.tile([65, 1024], F32, tag="pv", bufs=1, name=f"pv{b}{j}")[:]
                qc0 = b * S + j * 512

                def pv_mms(t, pt):
                    off = max(0, 128 * (t - 4 * j))
                    w = 512 - off
                    bt = b * NKT + t
                    nc.tensor.matmul(
                        pv[0:65, ds(off, w)], vaug[:, bt, 0:65], pt[:, 0, off:512],
                        start=(t == 0), stop=(t == ntk - 1),
                    )
                    nc.tensor.matmul(
                        pv[0:65, ds(512 + off, w)], vaug[:, bt, 65:130], pt[:, 1, off:512],
                        start=(t == 0), stop=(t == ntk - 1),
                    )

                prev = None  # software-pipeline: PV(t-1) issues after scores(t)
                for t in range(ntk):
                    off = max(0, 128 * (t - 4 * j))
                    w = 512 - off
                    diag = t >= 4 * j
                    sc = ps.tile([128, 2, 512], F32, tag="sc", bufs=2, name="sc")
                    pt = work.tile([128, 2, 512], BF, tag="ptile", bufs=3, name="pt")
                    kc = b * S + t * 128
                    # 4 concurrent quadrant matmuls: (head h) x (k-subtile s)
                    # occupy all four 64x64 PE tile groups
                    for h in range(2):
                        for s2 in range(2):
                            nc.tensor.matmul(
                                sc[64 * s2:64 * s2 + 64, h, off:512],
                                kt[64 * h:64 * h + 64, ds(kc + 64 * s2, 64)],
                                qt[64 * h:64 * h + 64, ds(qc0 + off, w)],
                                start=True, stop=not diag,
                            )
                    if diag:
                        # accumulate -240 onto sub-diagonal (q<k) positions;
                        # exp then flushes them to ~0 without a mask multiply
                        for h in range(2):
                            for s2 in range(2):
                                nc.tensor.matmul(
                                    sc[64 * s2:64 * s2 + 64, h, off:off + 128],
                                    mi64s[64 * h:64 * h + 64, :],
                                    steps[64 * h:64 * h + 64, s2, :],
                                    start=False, stop=True,
                                )
                    nc.scalar.activation(
                        pt[:, :, off:512], sc[:, :, off:512], EXPFN, scale=0.125,
                    )
                    # spread filler pieces evenly over remaining slots; put one
                    # before PV(t-1) to cover its wait on exp(t-1)
                    quota = (len(fillers) + ntk - t - 1) // (ntk - t)
                    if quota and fillers:
                        fillers.pop(0)()
                        quota -= 1
                    if prev is not None:
                        pv_mms(*prev)
                    for _ in range(quota):
                        if fillers:
                            fillers.pop(0)()
                    prev = (t, pt)
                while fillers:
                    fillers.pop(0)()
                pv_mms(*prev)
                # normalize by the denominator row (pv row 64)
                oc = ds(b * S + j * 512, 512)
                if last:
                    # half-width pipelined chain to cut the tail latency
                    for h in range(2):
                        hs = ds(h * 512, 512)
                        lbuf = work.tile([1, 512], F32, tag="lbufh", bufs=2, name="lbuf")
                        rbuf = work.tile([1, 512], F32, tag="rbufh", bufs=2, name="rbuf")
                        rb = work.tile([64, 512], F32, tag="rbh2", bufs=2, name="rb")
                        nc.vector.tensor_copy(lbuf[:], pv[64:65, hs])
                        nc.vector.reciprocal_approx_fast(rbuf[:], lbuf[:])
                        nc.gpsimd.partition_broadcast(rb[:], rbuf[:], channels=64)
                        nc.vector.tensor_mul(
                            attnT[64 * h:64 * h + 64, oc], pv[0:64, hs], rb[:],
                        )
                else:
                    lbuf = work.tile([1, 1024], F32, tag="lbuf", bufs=2, name="lbuf")
                    rbuf = work.tile([1, 1024], F32, tag="rbuf", bufs=2, name="rbuf")
                    nc.vector.tensor_copy(lbuf[:], pv[64:65, :])
                    nc.vector.reciprocal_approx_fast(rbuf[:], lbuf[:])
                    rb = work.tile([64, 1024], F32, tag="rb", bufs=2, name="rb")
                    nc.gpsimd.partition_broadcast(rb[:], rbuf[:], channels=64)
                    nc.vector.tensor_mul(attnT[0:64, oc], pv[0:64, 0:512], rb[:, 0:512])
                    nc.vector.tensor_mul(attnT[64:128, oc], pv[0:64, 512:1024], rb[:, 512:1024])

            def oproj_streamed(b, j):
                # split the final store so DMA overlaps the last copies
                oc = ds(b * S + j * 512, 512)
                ost = work.tile([128, 8, 512], BF, tag="ostage", bufs=2, name="ost")
                od = out_d[b].rearrange("(ec p) s -> p ec s", p=128)
                for e in range(8):
                    op = ps.tile([128, 512], F32, tag="work", bufs=2, name="op")
                    nc.tensor.matmul(
                        op[:], wos[:, ts(e, 128)], attnT[:, oc],
                        start=True, stop=True,
                    )
                    if e % 2 == 1:
                        nc.scalar.copy(ost[:, e, :], op[:])
                    else:
                        nc.vector.tensor_copy(ost[:, e, :], op[:])
                    if e == 3:
                        nc.sync.dma_start(out=od[:, 0:4, ts(j, 512)], in_=ost[:, 0:4, :])
                nc.sync.dma_start(out=od[:, 4:8, ts(j, 512)], in_=ost[:, 4:8, :])

            # ---- schedule ----
            # P(c): projection chunk c (c = b*4 + j); A(b,j): attention chunk;
            # O(b,j): output projection of chunk (b,j), emitted >=1 chunk after
            # its attention so the normalize has finished.  Large A chunks sit
            # mid-kernel where P/O filler is plentiful; 4-tile chunks open and
            # close the kernel so the serial head/tail is short.
            P = proj_pieces
            O = oproj_pieces
            for p in P(0):
                p()
            attn_chunk(0, 0, P(1))
            attn_chunk(0, 1, P(2))
            attn_chunk(0, 2, P(3))
            attn_chunk(0, 3, P(4) + P(5))
            attn_chunk(1, 1, P(6) + O(0, 0) + O(0, 1))
            attn_chunk(1, 2, P(7) + O(0, 2))
            attn_chunk(1, 3, O(0, 3) + O(1, 1))
            attn_chunk(1, 0, O(1, 2, tail=True) + O(1, 3, tail=True), last=True)
            oproj_streamed(1, 0)

    nc.compile()
    _nc_cache[key] = nc
    return nc


def make_in_maps(x, token_positions, wq, wk, wv, wo):
    bf = ml_dtypes.bfloat16
    xT = np.ascontiguousarray(
        np.asarray(x, np.float32).transpose(2, 0, 1).reshape(D, BS)
    ).astype(bf)
    pos = np.asarray(token_positions, np.float64)
    inv_freq = THETA ** (-(2.0 * np.arange(HD // 2, dtype=np.float64) / HD))
    ang = pos[:, None] * inv_freq[None, :]          # [S, 32]
    cos = np.cos(ang).astype(np.float32)
    sin = np.sin(ang).astype(np.float32)
    p = np.arange(128)
    idx = (p % HD) // 2
    cosT = np.ascontiguousarray(cos[:, idx].T).astype(bf)             # [128, S]
    sinT = np.ascontiguousarray(sin[:, idx].T).astype(bf)

    wq = np.asarray(wq, np.float32)
    wk = np.asarray(wk, np.float32)
    wv = np.asarray(wv, np.float32)
    wo = np.asarray(wo, np.float32)

    permT = np.zeros((128, 128), np.float32)
    me = np.arange(0, 128, 2)
    permT[me + 1, me] = -1.0      # swapped[even m] = -raw[m+1]
    permT[me, me + 1] = 1.0       # swapped[odd m]  = +raw[m-1]
    permT = permT.astype(bf)

    pp_ = np.arange(128)
    mm_ = np.arange(64)
    maskI = (-240.0 * ((pp_[:, None] % 64) == mm_[None, :])).astype(np.float32).astype(bf)
    qq_ = np.arange(128)
    stepU = np.stack(
        [(pp_[:, None] % 64) + 64 * s > qq_[None, :] for s in range(2)], axis=1
    ).astype(np.float32).astype(bf)   # [128, 2, 128]

    in_maps = []
    for c in range(NCORES):
        rows = slice(c * EL, (c + 1) * EL)
        in_maps.append({
            "xT": xT,
            "cosT": cosT,
            "sinT": sinT,
            "wqT": np.ascontiguousarray(wq[rows, :].T).astype(bf),
            "wkT": np.ascontiguousarray(wk[rows, :].T).astype(bf),
            "permT": permT,
            "wvT": np.ascontiguousarray(wv[rows, :].T).astype(bf),
            "woT": np.ascontiguousarray(wo[:, rows].T).astype(bf),
            "maskI": maskI,
            "stepU": stepU,
        })
    return in_maps


def unshard(results):
    acc = np.zeros((B, D, S), np.float32)
    for r in results:
        acc += np.asarray(r["out"], np.float32)
    return np.ascontiguousarray(acc.transpose(0, 2, 1))


def kernel(x, token_positions, wq, wk, wv, wo):
    from concourse.bass_utils import run_bass_kernel_spmd

    nc = build_nc(debug=False)
    in_maps = make_in_maps(x, token_positions, wq, wk, wv, wo)
    res = run_bass_kernel_spmd(nc, in_maps, core_ids=list(range(NCORES)))
    return unshard(res.results)


if __name__ == "__main__":
    # smoke test with random data
    rng = np.random.default_rng(0)
    x = rng.standard_normal((B, S, D), dtype=np.float32)
    tp = np.arange(S, dtype=np.int32)
    ws = [rng.standard_normal((D, D), dtype=np.float32) * 0.02 for _ in range(4)]
    out = kernel(x, tp, *ws)
    print(out.shape, out.dtype)


# revision 72
# speedup vs baseline: 1.0129x; 1.0129x over previous
"""Distributed causal attention (RoPE, QKV/out projections) on 8 TRN2 NeuronCores.

Sharding: tensor-parallel over heads. H=16 heads -> 2 heads per core.
Each core:
  - reads the full (transposed, bf16) activations xT [D, B*S]
  - computes qT/kT for its 2 heads (column-sharded wq/wk), applies RoPE
  - computes v in natural [s, hd] orientation (for the PV matmul lhsT),
    augmented with a ones-column so PV also produces the softmax denominator
  - flash-style causal attention with scores kept transposed [sk, sq] so
    softmax reduction runs on the TensorEngine via the ones-column trick
  - row-sharded output projection -> partial output [B, D, S]
Host sums the 8 partials and transposes back to [B, S, D].

Schedule: all QKV-projection and output-projection work is emitted as
"filler" pieces interleaved into the attention tile loops, so the PE has
dense work while ACT streams the softmax exps.  Chunk order places the
large (high-j) attention chunks mid-kernel where filler is plentiful and
small 4-tile chunks at the start/end.
"""

import numpy as np
import ml_dtypes

import concourse.mybir as mybir
from concourse import bacc
import concourse.tile as tile
from concourse.bass import ts, ds

B, S, D, H, HD = 2, 2048, 1024, 16, 64
NCORES = 8
HL = H // NCORES            # heads per core = 2
EL = HL * HD                # local e-dims per core = 128
BS = B * S                  # 4096
DCH = D // 128              # 8 contraction chunks
NCHUNK = BS // 512          # 8 projection chunks (both batches)
SQJ = S // 512              # 4 q-chunks per batch
NKT = S // 128              # 16 k-tiles per batch
THETA = 10000.0
BF = mybir.dt.bfloat16
F32 = mybir.dt.float32
FP8 = mybir.dt.float8e4
DR = mybir.MatmulPerfMode.DoubleRow
EXPFN = mybir.ActivationFunctionType.Exp
CPYFN = mybir.ActivationFunctionType.Copy
WSCALE = 16.0               # fp8 weights are stored x16; undone in rope/v-copy

_nc_cache = {}


def build_nc(debug=False):
    key = bool(debug)
    if key in _nc_cache:
        return _nc_cache[key]
    nc = bacc.Bacc("TRN2", target_bir_lowering=False, debug=debug, num_devices=NCORES)

    xT_d = nc.dram_tensor("xT", [D, BS], BF, kind="ExternalInput")
    cos_d = nc.dram_tensor("cosT", [128, S], BF, kind="ExternalInput")
    sin_d = nc.dram_tensor("sinT", [128, S], BF, kind="ExternalInput")
    wq_d = nc.dram_tensor("wqT", [D, EL], BF, kind="ExternalInput")
    wk_d = nc.dram_tensor("wkT", [D, EL], BF, kind="ExternalInput")
    perm_d = nc.dram_tensor("permT", [128, 128], BF, kind="ExternalInput")
    wv_d = nc.dram_tensor("wvT", [D, EL], BF, kind="ExternalInput")
    wo_d = nc.dram_tensor("woT", [EL, D], BF, kind="ExternalInput")
    mi_d = nc.dram_tensor("maskI", [128, 64], BF, kind="ExternalInput")
    su_d = nc.dram_tensor("stepU", [128, 2, 128], BF, kind="ExternalInput")
    out_d = nc.dram_tensor("out", [B, D, S], BF, kind="ExternalOutput")

    with tile.TileContext(nc) as tc:
        with (
            tc.tile_pool(name="sb", bufs=1) as sb,
            tc.tile_pool(name="work", bufs=2) as work,
            tc.tile_pool(name="ps", bufs=1, space="PSUM") as ps,
        ):
            # ---- persistent SBUF tensors ----
            xts = sb.tile([128, DCH, BS], BF)      # x transposed, d on partitions
            wqs = sb.tile([128, DCH, EL], BF)
            wks = sb.tile([128, DCH, EL], BF)
            perms = sb.tile([128, 128], BF)
            wvs = sb.tile([128, DCH, EL], BF)
            wos = sb.tile([128, D], BF)
            coss = sb.tile([128, S], BF)
            sins = sb.tile([128, S], BF)
            qt = sb.tile([128, BS], BF)
            kt = sb.tile([128, BS], BF)
            vaug = sb.tile([128, B * NKT, 130], BF)  # per k-tile: [v_h0|1|v_h1|1]
            attnT = sb.tile([128, BS], BF)           # normalized attn out, heads stacked

            # ---- input DMAs: weights + first x column groups spread across
            # queues so chunk 0's projection can start almost immediately ----
            mi64s = sb.tile([128, 64], BF)       # -240 * [p%64 == m]
            steps = sb.tile([128, 2, 128], BF)   # [(p%64) + 64s > q]

            for k in range(DCH):
                nc.sync.dma_start(out=wqs[:, k, :], in_=wq_d[ts(k, 128), :])
            nc.gpsimd.dma_start(out=wks[:], in_=wk_d[:, :].rearrange("(k p) e -> p k e", p=128))

            # x loads in three granularities: chunk-0 columns first (unblocks
            # the first projection fast), then the rest of batch 0, then
            # batch 1 — contiguous per-partition rows keep descriptors large.
            def xcols(k, lo, n, eng):
                eng.dma_start(
                    out=xts[:, k, ds(lo, n)], in_=xT_d[ts(k, 128), ds(lo, n)]
                )

            for k in range(DCH):
                xcols(k, 0, 512, (nc.sync, nc.gpsimd, nc.scalar)[k % 3])
            nc.sync.dma_start(out=perms[:], in_=perm_d[:, :])
            nc.sync.dma_start(out=mi64s[:], in_=mi_d[:, :])
            nc.sync.dma_start(out=steps[:], in_=su_d[:, :, :])
            for k in range(DCH):
                xcols(k, 512, 512, (nc.gpsimd, nc.sync, nc.scalar)[k % 3])
            nc.gpsimd.dma_start(out=coss[:], in_=cos_d[:, :])
            nc.gpsimd.dma_start(out=sins[:], in_=sin_d[:, :])
            for k in range(DCH):
                xcols(k, 1024, 1024, (nc.sync, nc.gpsimd)[k % 2])
            nc.gpsimd.dma_start(out=wvs[:], in_=wv_d[:, :].rearrange("(k p) e -> p k e", p=128))
            for k in range(DCH):
                xcols(k, S, S, (nc.sync, nc.gpsimd)[k % 2])
            nc.gpsimd.dma_start(out=wos[:], in_=wo_d[:, :])

            # ones columns for the PV denominator rows (full memset also marks
            # the tensor initialized for the simulator's strided-AP reads)
            nc.gpsimd.memset(vaug[:], 1.0)
            ones64 = sb.tile([1, 64], BF)
            nc.vector.memset(ones64[:], 1.0)
            scratch1 = sb.tile([128, 1], F32)
            nc.vector.memset(scratch1[:], 0.0)
            nc.scalar.activation(scratch1[:], scratch1[:], EXPFN)  # preload exp table

            # ---- QKV projection pieces for one 512-col chunk of B*S.
            # Returns a list of closures, each a contiguous PE work unit that
            # can be dropped into an attention tile loop as filler. ----
            def proj_pieces(c):
                cs = ds(c * 512, 512)
                scol = ds((c % SQJ) * 512, 512)     # position columns within batch
                pieces = []

                def qk(wtile, rot, nm):
                    st = {}

                    def mms():
                        pp = ps.tile([128, 512], F32, tag="work", bufs=2, name=f"pp{nm}{c}")
                        for k in range(DCH):
                            nc.tensor.matmul(
                                pp[:], wtile[:, k, :], xts[:, k, cs],
                                start=(k == 0), stop=(k == DCH - 1),
                            )
                        raw = work.tile([128, 512], BF, tag="rawt", bufs=2, name="raw")
                        nc.vector.tensor_copy(raw[:], pp[:])
                        st["raw"] = raw

                    def ropep():
                        raw = st["raw"]
                        sp2 = ps.tile([128, 512], F32, tag="work", bufs=2, name="sp2")
                        nc.tensor.matmul(sp2[:], perms[:], raw[:], start=True, stop=True)
                        rtmp = work.tile([128, 512], BF, tag="ropetmp", bufs=2, name="rtmp")
                        nc.vector.tensor_mul(rot[:, cs], raw[:], coss[:, scol])
                        nc.vector.tensor_mul(rtmp[:], sp2[:], sins[:, scol])
                        nc.vector.tensor_add(rot[:, cs], rot[:, cs], rtmp[:])

                    return [mms, ropep]

                pieces += qk(wqs, qt, "q")
                pieces += qk(wks, kt, "k")

                def vtile(st4):
                    def go():
                        t128 = c * 4 + st4
                        vp = ps.tile([128, 512], F32, tag="work", bufs=2, name=f"vp{t128}")
                        for k in range(DCH):
                            nc.tensor.matmul(
                                vp[:, 0:128], xts[:, k, ds(t128 * 128, 128)], wvs[:, k, :],
                                start=(k == 0), stop=(k == DCH - 1),
                            )
                        dst = vaug[:, t128, :].rearrange("p (g y) -> p g y", g=2)[:, :, 0:64]
                        src = vp[:, 0:128].rearrange("p (g y) -> p g y", g=2)
                        if st4 % 2 == 0:
                            nc.scalar.copy(dst, src)
                        else:
                            nc.vector.tensor_copy(dst, src)
                    return go

                pieces += [vtile(st4) for st4 in range(4)]
                return pieces

            # ---- output projection pieces for one (batch, q-chunk) ----
            def oproj_pieces(b, j, tail=False):
                oc = ds(b * S + j * 512, 512)
                ost = work.tile([128, 8, 512], BF, tag="ostage", bufs=2, name="ost")

                def piece(e):
                    def go():
                        op = ps.tile([128, 512], F32, tag="work", bufs=2, name="op")
                        nc.tensor.matmul(
                            op[:], wos[:, ts(e, 128)], attnT[:, oc],
                            start=True, stop=True,
                        )
                        # tail chunks alternate copy engines so the 2-bank op
                        # ring drains at two-copies-in-flight rate; elsewhere
                        # keep ACT exp-only
                        act = (e % 2 == 1) if tail else False
                        if act:
                            nc.scalar.copy(ost[:, e, :], op[:])
                        else:
                            nc.vector.tensor_copy(ost[:, e, :], op[:])
                    return go

                def store():
                    nc.sync.dma_start(
                        out=out_d[b].rearrange("(ec p) s -> p ec s", p=128)[:, :, ts(j, 512)],
                        in_=ost[:],
                    )
                return [piece(e) for e in range(8)] + [store]

            # ---- causal attention for one (batch, 512-wide q-chunk), with
            # filler pieces spread across the tile loop ----
            def attn_chunk(b, j, fillers=(), last=False):
                fillers = list(fillers)
                ntk = 4 * (j + 1)
                pv = ps.tile([65, 1024], F32, tag="pv", bufs=1, name=f"pv{b}{j}")[:]
                qc0 = b * S + j * 512

                def pv_mms(t, pt):
                    off = max(0, 128 * (t - 4 * j))
                    w = 512 - off
                    bt = b * NKT + t
                    nc.tensor.matmul(
                        pv[0:65, ds(off, w)], vaug[:, bt, 0:65], pt[:, 0, off:512],
                        start=(t == 0), stop=(t == ntk - 1),
                    )
                    nc.tensor.matmul(
                        pv[0:65, ds(512 + off, w)], vaug[:, bt, 65:130], pt[:, 1, off:512],
                        start=(t == 0), stop=(t == ntk - 1),
                    )

                prev = None  # software-pipeline: PV(t-1) issues after scores(t)
                for t in range(ntk):
                    off = max(0, 128 * (t - 4 * j))
                    w = 512 - off
                    diag = t >= 4 * j
                    sc = ps.tile([128, 2, 512], F32, tag="sc", bufs=2, name="sc")
                    pt = work.tile([128, 2, 512], BF, tag="ptile", bufs=3, name="pt")
                    kc = b * S + t * 128
                    # 4 concurrent quadrant matmuls: (head h) x (k-subtile s)
                    # occupy all four 64x64 PE tile groups
                    for h in range(2):
                        for s2 in range(2):
                            nc.tensor.matmul(
                                sc[64 * s2:64 * s2 + 64, h, off:512],
                                kt[64 * h:64 * h + 64, ds(kc + 64 * s2, 64)],
                                qt[64 * h:64 * h + 64, ds(qc0 + off, w)],
                                start=True, stop=not diag,
                            )
                    if diag:
                        # accumulate -240 onto sub-diagonal (q<k) positions;
                        # exp then flushes them to ~0 without a mask multiply
                        for h in range(2):
                            for s2 in range(2):
                                nc.tensor.matmul(
                                    sc[64 * s2:64 * s2 + 64, h, off:off + 128],
                                    mi64s[64 * h:64 * h + 64, :],
                                    steps[64 * h:64 * h + 64, s2, :],
                                    start=False, stop=True,
                                )
                    nc.scalar.activation(
                        pt[:, :, off:512], sc[:, :, off:512], EXPFN, scale=0.125,
                    )
                    # spread filler pieces evenly over remaining slots; put one
                    # before PV(t-1) to cover its wait on exp(t-1)
                    quota = (len(fillers) + ntk - t - 1) // (ntk - t)
                    if quota and fillers:
                        fillers.pop(0)()
                        quota -= 1
                    if prev is not None:
                        pv_mms(*prev)
                    for _ in range(quota):
                        if fillers:
                            fillers.pop(0)()
                    prev = (t, pt)
                while fillers:
                    fillers.pop(0)()
                pv_mms(*prev)
                # normalize by the denominator row (pv row 64)
                oc = ds(b * S + j * 512, 512)
                if last:
                    # half-width pipelined chain to cut the tail latency
                    for h in range(2):
                        hs = ds(h * 512, 512)
                        lbuf = work.tile([1, 512], F32, tag="lbufh", bufs=2, name="lbuf")
                        rbuf = work.tile([1, 512], F32, tag="rbufh", bufs=2, name="rbuf")
                        rb = work.tile([64, 512], F32, tag="rbh2", bufs=2, name="rb")
                        nc.vector.tensor_copy(lbuf[:], pv[64:65, hs])
                        nc.vector.reciprocal_approx_fast(rbuf[:], lbuf[:])
                        nc.gpsimd.partition_broadcast(rb[:], rbuf[:], channels=64)
                        nc.vector.tensor_mul(
                            attnT[64 * h:64 * h + 64, oc], pv[0:64, hs], rb[:],
                        )
                else:
                    lbuf = work.tile([1, 1024], F32, tag="lbuf", bufs=2, name="lbuf")
                    rbuf = work.tile([1, 1024], F32, tag="rbuf", bufs=2, name="rbuf")
                    nc.vector.tensor_copy(lbuf[:], pv[64:65, :])
                    nc.vector.reciprocal_approx_fast(rbuf[:], lbuf[:])
                    rb = work.tile([64, 1024], F32, tag="rb", bufs=2, name="rb")
                    nc.gpsimd.partition_broadcast(rb[:], rbuf[:], channels=64)
                    nc.vector.tensor_mul(attnT[0:64, oc], pv[0:64, 0:512], rb[:, 0:512])
                    nc.vector.tensor_mul(attnT[64:128, oc], pv[0:64, 512:1024], rb[:, 512:1024])

            def oproj_streamed(b, j):
                # split the final store so DMA overlaps the last copies
                oc = ds(b * S + j * 512, 512)
                ost = work.tile([128, 8, 512], BF, tag="ostage", bufs=2, name="ost")
                od = out_d[b].rearrange("(ec p) s -> p ec s", p=128)
                for e in range(8):
                    op = ps.tile([128, 512], F32, tag="work", bufs=2, name="op")
                    nc.tensor.matmul(
                        op[:], wos[:, ts(e, 128)], attnT[:, oc],
                        start=True, stop=True,
                    )
                    if e % 2 == 1:
                        nc.scalar.copy(ost[:, e, :], op[:])
                    else:
                        nc.vector.tensor_copy(ost[:, e, :], op[:])
                    if e == 3:
                        nc.sync.dma_start(out=od[:, 0:4, ts(j, 512)], in_=ost[:, 0:4, :])
                nc.sync.dma_start(out=od[:, 4:8, ts(j, 512)], in_=ost[:, 4:8, :])

            # ---- schedule ----
            # P(c): projection chunk c (c = b*4 + j); A(b,j): attention chunk;
            # O(b,j): output projection of chunk (b,j), emitted >=1 chunk after
            # its attention so the normalize has finished.  Large A chunks sit
            # mid-kernel where P/O filler is plentiful; 4-tile chunks open and
            # close the kernel so the serial head/tail is short.
            P = proj_pieces
            O = oproj_pieces
            for p in P(0):
                p()
            attn_chunk(0, 0, P(1))
            attn_chunk(0, 1, P(2))
            attn_chunk(0, 2, P(3))
            attn_chunk(0, 3, P(4) + P(5))
            attn_chunk(1, 1, P(6) + O(0, 0, tail=True) + O(0, 1, tail=True))
            attn_chunk(1, 2, P(7) + O(0, 2))
            attn_chunk(1, 3, O(0, 3) + O(1, 1))
            attn_chunk(1, 0, O(1, 2, tail=True) + O(1, 3, tail=True), last=True)
            oproj_streamed(1, 0)

    nc.compile()
    _nc_cache[key] = nc
    return nc


def make_in_maps(x, token_positions, wq, wk, wv, wo):
    bf = ml_dtypes.bfloat16
    xT = np.ascontiguousarray(
        np.asarray(x, np.float32).transpose(2, 0, 1).reshape(D, BS)
    ).astype(bf)
    pos = np.asarray(token_positions, np.float64)
    inv_freq = THETA ** (-(2.0 * np.arange(HD // 2, dtype=np.float64) / HD))
    ang = pos[:, None] * inv_freq[None, :]          # [S, 32]
    cos = np.cos(ang).astype(np.float32)
    sin = np.sin(ang).astype(np.float32)
    p = np.arange(128)
    idx = (p % HD) // 2
    cosT = np.ascontiguousarray(cos[:, idx].T).astype(bf)             # [128, S]
    sinT = np.ascontiguousarray(sin[:, idx].T).astype(bf)

    wq = np.asarray(wq, np.float32)
    wk = np.asarray(wk, np.float32)
    wv = np.asarray(wv, np.float32)
    wo = np.asarray(wo, np.float32)

    permT = np.zeros((128, 128), np.float32)
    me = np.arange(0, 128, 2)
    permT[me + 1, me] = -1.0      # swapped[even m] = -raw[m+1]
    permT[me, me + 1] = 1.0       # swapped[odd m]  = +raw[m-1]
    permT = permT.astype(bf)

    pp_ = np.arange(128)
    mm_ = np.arange(64)
    maskI = (-240.0 * ((pp_[:, None] % 64) == mm_[None, :])).astype(np.float32).astype(bf)
    qq_ = np.arange(128)
    stepU = np.stack(
        [(pp_[:, None] % 64) + 64 * s > qq_[None, :] for s in range(2)], axis=1
    ).astype(np.float32).astype(bf)   # [128, 2, 128]

    in_maps = []
    for c in range(NCORES):
        rows = slice(c * EL, (c + 1) * EL)
        in_maps.append({
            "xT": xT,
            "cosT": cosT,
            "sinT": sinT,
            "wqT": np.ascontiguousarray(wq[rows, :].T).astype(bf),
            "wkT": np.ascontiguousarray(wk[rows, :].T).astype(bf),
            "permT": permT,
            "wvT": np.ascontiguousarray(wv[rows, :].T).astype(bf),
            "woT": np.ascontiguousarray(wo[:, rows].T).astype(bf),
            "maskI": maskI,
            "stepU": stepU,
        })
    return in_maps


def unshard(results):
    acc = np.zeros((B, D, S), np.float32)
    for r in results:
        acc += np.asarray(r["out"], np.float32)
    return np.ascontiguousarray(acc.transpose(0, 2, 1))


def kernel(x, token_positions, wq, wk, wv, wo):
    from concourse.bass_utils import run_bass_kernel_spmd

    nc = build_nc(debug=False)
    in_maps = make_in_maps(x, token_positions, wq, wk, wv, wo)
    res = run_bass_kernel_spmd(nc, in_maps, core_ids=list(range(NCORES)))
    return unshard(res.results)


if __name__ == "__main__":
    # smoke test with random data
    rng = np.random.default_rng(0)
    x = rng.standard_normal((B, S, D), dtype=np.float32)
    tp = np.arange(S, dtype=np.int32)
    ws = [rng.standard_normal((D, D), dtype=np.float32) * 0.02 for _ in range(4)]
    out = kernel(x, tp, *ws)
    print(out.shape, out.dtype)
